# revision 2
# baseline (speedup 1.0000x reference)
"""HardTripletLoss (non-hardest branch) on 8 TRN2 NeuronCores.

Math:  loss = mean_{i!=j} relu(d_pos[i] - pdist[i,j] + margin)
  pdist[i,j] = ||x_i||^2 + ||y_j||^2 - 2 x_i.y_j ,  d_pos = diag(pdist)
  =>  relu(G[i,j] + a[i] - b[j])  with  G = 2 x y^T,
      a[i] = ||y_i||^2 - 2 x_i.y_i + margin,  b[j] = ||y_j||^2.
Diagonal (i==j) evaluates to exactly relu(margin) = margin, so we compute the
full unmasked sum and subtract N*margin on the host.

Sharding: x rows split across 8 cores (data parallel), y replicated.
Per core: bf16 matmul G-tiles into PSUM; epilogue split between
 - DVE:  sum_j max(G+a, b) per row (fused scalar_tensor_tensor w/ accum),
         then the known Sum_b is subtracted on the host
           (uses relu(z-b) = max(z, b) - b),
 - ACT:  PE folds -b into PSUM via a K=2 ones x [b_hi;b_lo] matmul (bf16
         hi/lo split keeps b exact to ~1e-3), then activation(Relu, bias=a)
         with free-dim accumulate.
Row-partial sums land in a [128, 64] tile per core; host reduces in f64.
"""

import sys

if "/opt/trn_rl_repo" not in sys.path:
    sys.path.insert(0, "/opt/trn_rl_repo")

import numpy as np

N, D = 8192, 128
NCORES = 8
SH = N // NCORES          # 1024 x-rows per core
MT = SH // 128            # 8 m-tiles (128 rows each)
NT2 = N // 1024           # 8 double-tiles (1024 cols each)
NYT = N // 128            # 64 y row-tiles
MARGIN = 0.2
# double-tile (m,n) handled by ACT when (m*NT2+n) % ACT_MOD == 0, else DVE
ACT_MOD = 2

_cache = {}


def _build():
    import concourse.bass as bass
    import concourse.mybir as mybir
    from concourse import bacc
    from concourse.tile import TileContext
    from concourse.bass import ts

    f32 = mybir.dt.float32
    bf16 = mybir.dt.bfloat16
    Alu = mybir.AluOpType
    Act = mybir.ActivationFunctionType

    nc = bacc.Bacc()
    xb = nc.declare_dram_parameter("xb", [SH, D], bf16, isOutput=False)
    yb = nc.declare_dram_parameter("yb", [N, D], bf16, isOutput=False)
    xf = nc.declare_dram_parameter("xf", [SH, D], f32, isOutput=False)
    ylf = nc.declare_dram_parameter("ylf", [SH, D], f32, isOutput=False)
    yf = nc.declare_dram_parameter("yf", [N, D], f32, isOutput=False)
    out_res = nc.declare_dram_parameter("res", [128, MT * NT2], f32, isOutput=True)
    out_b = nc.declare_dram_parameter("bvec", [1, N], f32, isOutput=True)

    s_b = nc.dram_tensor("s_b", [1, N], f32)
    s_hi = nc.dram_tensor("s_hi", [1, N], bf16)
    s_lo = nc.dram_tensor("s_lo", [1, N], bf16)

    yf3 = yf.rearrange("(t p) d -> p t d", p=128)
    xf3 = xf.rearrange("(t p) d -> p t d", p=128)
    ylf3 = ylf.rearrange("(t p) d -> p t d", p=128)

    with TileContext(nc) as tc:
        with (
            tc.tile_pool(name="big", bufs=1) as big,
            tc.tile_pool(name="ld", bufs=4) as ld,
            tc.tile_pool(name="work", bufs=3) as work,
            tc.tile_pool(name="ps", bufs=3, space="PSUM") as ps,
        ):
            yT = big.tile([128, N], bf16)
            xT = big.tile([128, SH], bf16)
            bbs = [
                big.tile([128, 1024], f32, tag=f"bb{n}", name=f"bb{n}")
                for n in range(NT2)
            ]
            rhs2 = big.tile([2, N], bf16)         # [b_hi ; b_lo]
            nones = big.tile([2, 128], bf16)      # -1, fold weights
            yy = big.tile([128, NYT], f32)        # ||y_j||^2, row-major tile layout
            hi = big.tile([128, NYT], bf16)
            hi32 = big.tile([128, NYT], f32)
            lo32 = big.tile([128, NYT], f32)
            lo = big.tile([128, NYT], bf16)
            z2 = big.tile([128, MT], f32)         # 2 x_i.y_i
            yyl = big.tile([128, MT], f32)        # ||y_i||^2, shard rows
            acol = big.tile([128, MT], f32)       # a per m-tile
            res = big.tile([128, MT * NT2], f32)

            # ---- transposed matmul operand loads (bf16, DMA transpose) ----
            for n in range(NT2):
                nc.sync.dma_start_transpose(
                    yT[:, n * 1024 : (n + 1) * 1024],
                    yb[n * 1024 : (n + 1) * 1024, :],
                )
            for m in range(MT):
                nc.sync.dma_start_transpose(xT[:, ts(m, 128)], xb[ts(m, 128), :])
            nc.vector.memset(nones[:], -1.0)

            # ---- row norms of y (split DVE/ACT), from f32 row-major tiles ----
            for t in range(NYT):
                yt = ld.tile([128, D], f32, tag="ld_y")
                nc.gpsimd.dma_start(yt[:], yf3[:, t, :])
                if t % 2 == 0:
                    scr = work.tile([128, D], f32, tag="sq_act")
                    nc.scalar.activation(
                        scr[:], yt[:], Act.Square,
                        accum_out=yy[:, t : t + 1],
                    )
                else:
                    scr = work.tile([128, D], f32, tag="sq_dve")
                    nc.vector.scalar_tensor_tensor(
                        out=scr[:], in0=yt[:], scalar=1.0, in1=yt[:],
                        op0=Alu.mult, op1=Alu.mult,
                        accum_out=yy[:, t : t + 1],
                    )

            # ---- a = yy_shard + margin - 2 x.y_shard ----
            for m in range(MT):
                xt = ld.tile([128, D], f32, tag="ld_x")
                yl = ld.tile([128, D], f32, tag="ld_yl")
                nc.gpsimd.dma_start(xt[:], xf3[:, m, :])
                nc.gpsimd.dma_start(yl[:], ylf3[:, m, :])
                scr = work.tile([128, D], f32, tag="z2_dve")
                nc.vector.scalar_tensor_tensor(
                    out=scr[:], in0=xt[:], scalar=2.0, in1=yl[:],
                    op0=Alu.mult, op1=Alu.mult,
                    accum_out=z2[:, m : m + 1],
                )
                scr2 = work.tile([128, D], f32, tag="yyl_act")
                nc.scalar.activation(
                    scr2[:], yl[:], Act.Square,
                    accum_out=yyl[:, m : m + 1],
                )
            for m in range(MT):
                nc.vector.scalar_tensor_tensor(
                    out=acol[:, m : m + 1], in0=yyl[:, m : m + 1], scalar=MARGIN,
                    in1=z2[:, m : m + 1], op0=Alu.add, op1=Alu.subtract,
                )

            # ---- b hi/lo split (bf16 + residual) ----
            nc.scalar.activation(hi[:], yy[:], Act.Copy)
            nc.scalar.activation(hi32[:], hi[:], Act.Copy)
            nc.vector.scalar_tensor_tensor(
                out=lo32[:], in0=yy[:], scalar=1.0, in1=hi32[:],
                op0=Alu.mult, op1=Alu.subtract,
            )
            nc.scalar.activation(lo[:], lo32[:], Act.Copy)

            # ---- relayout b via DRAM round-trip: (p,t) -> j = t*128+p ----
            # gpsimd (SWDGE) for compute-dependent DMAs
            nc.gpsimd.dma_start(s_b[0, :].rearrange("(t p) -> p t", p=128), yy[:])
            nc.gpsimd.dma_start(s_hi[0, :].rearrange("(t p) -> p t", p=128), hi[:])
            nc.gpsimd.dma_start(s_lo[0, :].rearrange("(t p) -> p t", p=128), lo[:])
            nc.gpsimd.dma_start(out_b[:], s_b[:])
            nc.gpsimd.dma_start(rhs2[0:1, :], s_hi[:])
            nc.gpsimd.dma_start(rhs2[1:2, :], s_lo[:])
            # partition-broadcast b into 8 x [128, 1024]
            for n in range(NT2):
                nc.gpsimd.dma_start(
                    bbs[n][:],
                    s_b[:, n * 1024 : (n + 1) * 1024].broadcast_to([128, 1024]),
                )

            # ---- main: G tiles + fused epilogue ----
            for m in range(MT):
                for n in range(NT2):
                    idx = m * NT2 + n
                    is_act = (idx % ACT_MOD) == 0
                    pt = ps.tile([128, 1024], f32, tag="g")
                    for h in range(2):
                        nc.tensor.matmul(
                            pt[:, h * 512 : (h + 1) * 512],
                            lhsT=xT[:, ts(m, 128)],
                            rhs=yT[:, n * 1024 + h * 512 : n * 1024 + (h + 1) * 512],
                            start=True, stop=not is_act,
                        )
                    if is_act:
                        for h in range(2):
                            nc.tensor.matmul(
                                pt[:, h * 512 : (h + 1) * 512],
                                lhsT=nones[:],
                                rhs=rhs2[:, n * 1024 + h * 512 : n * 1024 + (h + 1) * 512],
                                start=False, stop=True,
                            )
                        scr = work.tile([128, 1024], f32, tag="ep_act")
                        nc.scalar.activation(
                            scr[:], pt[:], Act.Relu,
                            bias=acol[:, m : m + 1],
                            accum_out=res[:, idx : idx + 1],
                        )
                    else:
                        scr = work.tile([128, 1024], f32, tag="ep_dve")
                        nc.vector.scalar_tensor_tensor(
                            out=scr[:], in0=pt[:], scalar=acol[:, m : m + 1],
                            in1=bbs[n][:],
                            op0=Alu.add, op1=Alu.max,
                            accum_out=res[:, idx : idx + 1],
                        )

            nc.gpsimd.dma_start(out_res[:], res[:])

    return nc


def _make_in_maps(x: np.ndarray, y: np.ndarray) -> list:
    import ml_dtypes

    x = np.ascontiguousarray(x, dtype=np.float32)
    y = np.ascontiguousarray(y, dtype=np.float32)
    yb = y.astype(ml_dtypes.bfloat16)
    in_maps = []
    for c in range(NCORES):
        sl = slice(c * SH, (c + 1) * SH)
        in_maps.append({
            "xb": (2.0 * x[sl]).astype(ml_dtypes.bfloat16),
            "yb": yb,
            "xf": x[sl],
            "ylf": y[sl],
            "yf": y,
        })
    return in_maps


def kernel(x: np.ndarray, y: np.ndarray) -> np.ndarray:
    from concourse.bass_utils import run_bass_kernel_spmd

    x = np.ascontiguousarray(x, dtype=np.float32)
    y = np.ascontiguousarray(y, dtype=np.float32)

    if "nc" not in _cache:
        nc = _build()
        if not nc.is_finalized():
            nc.finalize()
        _cache["nc"] = nc
    nc = _cache["nc"]

    out = run_bass_kernel_spmd(nc, _make_in_maps(x, y), list(range(NCORES)))
    results = out.results

    # host reduction (f64)
    total = 0.0
    for c in range(NCORES):
        total += np.asarray(results[c]["res"], dtype=np.float64).sum()
    b_dev = np.asarray(results[0]["bvec"], dtype=np.float64).reshape(N)
    # subtract Sum_b for every DVE tile (max-trick correction)
    bsum_tile = b_dev.reshape(NT2, 1024).sum(axis=1)
    for m in range(MT):
        for n in range(NT2):
            if (m * NT2 + n) % ACT_MOD != 0:
                total -= NCORES * 128.0 * bsum_tile[n]
    total -= float(N) * float(np.float32(MARGIN))
    return np.float32(total / (float(N) * float(N)))



# revision 7
# speedup vs baseline: 1.4981x; 1.4981x over previous
"""HardTripletLoss (non-hardest branch) on 8 TRN2 NeuronCores.

Math:  loss = mean_{i!=j} relu(d_pos[i] - pdist[i,j] + margin)
  pdist[i,j] = ||x_i||^2 + ||y_j||^2 - 2 x_i.y_j ,  d_pos = diag(pdist)
  =>  per-term: relu(G[i,j] + a[i] - b[j]) with G = 2 x y^T,
      a[i] = margin + b[i] - G[i,i],  b[j] = ||y_j||^2  (xx cancels).
Diagonal (i==j) evaluates to ~relu(margin) = margin; the full unmasked sum is
computed and N*margin subtracted on the host.

Sharding: x rows split across 8 cores, y replicated.  Inputs arrive
pre-transposed/scaled from the host (bf16): xT2 = (2 x_shard)^T [128,1024],
ylT = y_shard^T [128,1024] (bit-identical to the matching yT slice),
yT = y^T [128,8192].  ~2.5 MB HBM per core, all HWDGE.

Per core, per 1024-col group n (8 groups):
  sq_n   = square(yT_n) bf16                      (ACT/DVE split, preamble)
  b_bb_n = ones128^T @ sq_n -> PSUM [128,1024]    (b broadcast to all rows)
  for m in 0..7:  G2 = xT2_m^T @ yT_n -> PSUM  (216 ns / 512-col matmul)
    m even (DVE): sum_j max(G2 + a_m, b_bb_n)  (STT, accum)  [max-trick:
                  host subtracts 128*sum(b) per DVE chunk]
    m odd  (ACT): PE chains -ones128^T @ sq_n into the same PSUM (= G2 - b),
                  then relu(. + a_m) with accum.  Fold is the exact negation
                  of b_bb (same PE summation order), keeping both paths
                  consistent.
a-path: sqc = square(ylT); prod = xT2*ylT elementwise; z2bb/bbc via
ones-matmuls; a16[1,1024] = (bbc + margin) - z2bb on partition 0;
PE-transpose (lhsT=a16 chunk, rhs=[1,1] ones) -> acol [128,8] f32.
Host: loss = (sum(res) - NCORES*|DVE_MS|*128*sum(b) - N*margin) / N^2 in f64;
sum(b) is recomputed on host from the same bf16 quantization (the f32
summation-order drift is ~1e-7 relative, tolerance is 2e-2).
"""

import sys

if "/opt/trn_rl_repo" not in sys.path:
    sys.path.insert(0, "/opt/trn_rl_repo")

import numpy as np

N, D = 8192, 128
NCORES = 8
SH = N // NCORES          # 1024 x-rows per core
MT = SH // 128            # 8 m-tiles
NG = N // 1024            # 8 col groups of 1024
MARGIN = 0.2
# m-tile -> engine: even m = DVE (max-trick), odd m = ACT (PE-fold + relu).
# ACT m's run first in each col so the bb PSUM->SBUF DMA can complete.
DVE_MS = tuple(m for m in range(MT) if m % 2 == 0)
M_ORDER = [m for m in range(MT) if m not in DVE_MS] + list(DVE_MS)

_cache = {}


def _build():
    import concourse.mybir as mybir
    from concourse import bacc
    from concourse.tile import TileContext
    from concourse.bass import ts

    f32 = mybir.dt.float32
    bf16 = mybir.dt.bfloat16
    Alu = mybir.AluOpType
    Act = mybir.ActivationFunctionType

    nc = bacc.Bacc()
    xT_in = nc.declare_dram_parameter("xT2", [128, SH], bf16, isOutput=False)
    ylT_in = nc.declare_dram_parameter("ylT", [128, SH], bf16, isOutput=False)
    yT_in = nc.declare_dram_parameter("yT", [128, N], bf16, isOutput=False)
    out_res = nc.declare_dram_parameter("res", [128, MT * NG], f32, isOutput=True)

    with TileContext(nc) as tc:
        with (
            tc.tile_pool(name="big", bufs=1) as big,
            tc.tile_pool(name="work", bufs=3) as work,
            tc.tile_pool(name="ps", bufs=1, space="PSUM") as ps,
        ):
            yT = big.tile([128, N], bf16)
            xT = big.tile([128, SH], bf16)
            ylT = big.tile([128, SH], bf16)
            sq = big.tile([128, N], bf16)
            sqc = big.tile([128, SH], bf16)
            ones128 = big.tile([128, 128], bf16)
            negones = big.tile([128, 128], bf16)
            ones1 = big.tile([1, 1], bf16)
            prod = big.tile([128, SH], bf16)
            a16 = big.tile([1, SH], bf16)
            acol = big.tile([128, MT], f32)
            res = big.tile([128, MT * NG], f32)

            nc.vector.memset(ones128[:], 1.0)
            nc.vector.memset(negones[:], -1.0)
            nc.vector.memset(ones1[:], 1.0)

            nc.sync.dma_start(xT[:], xT_in[:])
            nc.sync.dma_start(ylT[:], ylT_in[:])
            for n in range(NG):
                nc.sync.dma_start(yT[:, ts(n, 1024)], yT_in[:, ts(n, 1024)])

            # ---- a-path (gates ACT epilogue via acol) ----
            nc.scalar.activation(sqc[:], ylT[:], Act.Square)
            nc.vector.scalar_tensor_tensor(
                out=prod[:], in0=xT[:], scalar=1.0, in1=ylT[:],
                op0=Alu.mult, op1=Alu.mult,
            )
            z2bb = ps.tile([128, 1024], f32, tag="bb")
            for h in range(2):
                nc.tensor.matmul(
                    z2bb[:, ts(h, 512)], lhsT=ones128[:],
                    rhs=prod[:, ts(h, 512)], start=True, stop=True,
                )
            bbc = ps.tile([128, 1024], f32, tag="g")
            for h in range(2):
                nc.tensor.matmul(
                    bbc[:, ts(h, 512)], lhsT=ones128[:],
                    rhs=sqc[:, ts(h, 512)], start=True, stop=True,
                )
            # a16 = (b + margin) - z2, partition 0 only (one PSUM input max)
            z2r = big.tile([1, SH], f32)
            nc.scalar.activation(z2r[0:1, :], z2bb[0:1, :], Act.Copy)
            nc.vector.scalar_tensor_tensor(
                out=a16[0:1, :], in0=bbc[0:1, :], scalar=MARGIN,
                in1=z2r[0:1, :], op0=Alu.add, op1=Alu.subtract,
            )
            # transpose a16 -> acol via PE
            tp = ps.tile([128, MT], f32, tag="tp")
            for m in range(MT):
                nc.tensor.matmul(
                    tp[:, m : m + 1], lhsT=a16[0:1, ts(m, 128)],
                    rhs=ones1[:], start=True, stop=True,
                )
                nc.scalar.activation(acol[:, m : m + 1], tp[:, m : m + 1], Act.Copy)

            # ---- squares for all col groups (ACT/DVE alternating) ----
            for n in range(NG):
                if n % 2 == 0:
                    nc.scalar.activation(
                        sq[:, ts(n, 1024)], yT[:, ts(n, 1024)], Act.Square
                    )
                else:
                    nc.vector.scalar_tensor_tensor(
                        out=sq[:, ts(n, 1024)], in0=yT[:, ts(n, 1024)],
                        scalar=1.0, in1=yT[:, ts(n, 1024)],
                        op0=Alu.mult, op1=Alu.mult,
                    )

            # ---- main loop ----
            for n in range(NG):
                bb = ps.tile([128, 1024], f32, tag="bb")
                for h in range(2):
                    nc.tensor.matmul(
                        bb[:, ts(h, 512)], lhsT=ones128[:],
                        rhs=sq[:, n * 1024 + h * 512 : n * 1024 + (h + 1) * 512],
                        start=True, stop=True,
                    )
                bbsb = work.tile([128, 1024], f32, tag="bbsb", bufs=2)
                if n % 2 == 0:
                    nc.scalar.activation(bbsb[:], bb[:], Act.Copy)
                else:
                    nc.vector.tensor_copy(out=bbsb[:], in_=bb[:])
                for m in M_ORDER:
                    idx = n * MT + m
                    is_dve = m in DVE_MS
                    pt = ps.tile([128, 1024], f32, tag="g")
                    for h in range(2):
                        nc.tensor.matmul(
                            pt[:, ts(h, 512)],
                            lhsT=xT[:, ts(m, 128)],
                            rhs=yT[:, n * 1024 + h * 512 : n * 1024 + (h + 1) * 512],
                            start=True, stop=is_dve,
                        )
                        if not is_dve:
                            nc.tensor.matmul(
                                pt[:, ts(h, 512)],
                                lhsT=negones[:],
                                rhs=sq[:, n * 1024 + h * 512 : n * 1024 + (h + 1) * 512],
                                start=False, stop=True,
                            )
                    if is_dve:
                        scr = work.tile([128, 1024], f32, tag="ep_dve")
                        nc.vector.scalar_tensor_tensor(
                            out=scr[:], in0=pt[:], scalar=acol[:, m : m + 1],
                            in1=bbsb[:], op0=Alu.add, op1=Alu.max,
                            accum_out=res[:, idx : idx + 1],
                        )
                    else:
                        scr = work.tile([128, 1024], f32, tag="ep_act")
                        nc.scalar.activation(
                            scr[:], pt[:], Act.Relu,
                            bias=acol[:, m : m + 1],
                            accum_out=res[:, idx : idx + 1],
                        )

            nc.sync.dma_start(out_res[:], res[:])

    return nc


def _make_in_maps(x: np.ndarray, y: np.ndarray) -> list:
    import ml_dtypes

    x = np.ascontiguousarray(x, dtype=np.float32)
    y = np.ascontiguousarray(y, dtype=np.float32)
    yb = y.astype(ml_dtypes.bfloat16)
    yT = np.ascontiguousarray(yb.T)
    in_maps = []
    for c in range(NCORES):
        sl = slice(c * SH, (c + 1) * SH)
        xT2 = np.ascontiguousarray((2.0 * x[sl]).astype(ml_dtypes.bfloat16).T)
        ylT = np.ascontiguousarray(yb[sl].T)
        in_maps.append({"xT2": xT2, "ylT": ylT, "yT": yT})
    return in_maps


def kernel(x: np.ndarray, y: np.ndarray) -> np.ndarray:
    from concourse.bass_utils import run_bass_kernel_spmd
    import ml_dtypes

    x = np.ascontiguousarray(x, dtype=np.float32)
    y = np.ascontiguousarray(y, dtype=np.float32)

    if "nc" not in _cache:
        nc = _build()
        if not nc.is_finalized():
            nc.finalize()
        _cache["nc"] = nc
    nc = _cache["nc"]

    out = run_bass_kernel_spmd(nc, _make_in_maps(x, y), list(range(NCORES)))
    results = out.results

    total = 0.0
    for c in range(NCORES):
        total += np.asarray(results[c]["res"], dtype=np.float64).sum()

    # host-side sum(b): same bf16 quantization as the device pipeline
    yb = y.astype(ml_dtypes.bfloat16).astype(np.float32)
    sq = np.square(yb).astype(ml_dtypes.bfloat16).astype(np.float64)
    sum_b = sq.sum()
    total -= NCORES * len(DVE_MS) * 128.0 * sum_b
    total -= float(N) * float(np.float32(MARGIN))
    return np.float32(total / (float(N) * float(N)))
